# revision 1
# baseline (speedup 1.0000x reference)
"""AFT-Full (Attention Free Transformer) kernel for Trainium2, 8 NeuronCores.

Model (per batch b):
    q = x @ Wq + bq;  k = x @ Wk + bk;  v = x @ Wv + bv
    out[i,d] = sigmoid(q)[i,d] * sum_j exp(B[i,j])*exp(k[j,d])*v[j,d]
                               / sum_j exp(B[i,j])*exp(k[j,d])

Sharding: data-parallel over batch (BS=8 -> 1 batch per core). pos_bias is
replicated (transposed on host so the contraction index j lands on SBUF
partitions). Matmul operands are bf16; accumulation is fp32 in PSUM.

Bias handling (no bias matmuls at all):
  - bk cancels exactly in the num/den ratio -> dropped.
  - bv enters linearly: ekv = ek * (v + bv), added on DVE in phase 1b.
  - bq is broadcast-added on DVE before the sigmoid in phase 1a.

Per-core device schedule:
  warmup:   a few dependency-free matmuls on memset tiles raise the PE HAM
            clock gate to full rate while the first DMAs land.
  phase 1a: q-projection (4 K-chunk matmuls into PSUM), sig = sigmoid(q+bq).
  phase 1b: k,v projections; ek = exp(k) (ACT -> bf16), ekv = ek*(v+bv)
            (DVE -> bf16), stored as X = [ekv | ek] per j-chunk.
  phase 2:  per output i-chunk: DMA bf16 tile of exp(B^T) (exp precomputed
            host-side on the replicated tensor, 4 i-chunks per DMA for
            contiguous runs), 32 accumulating matmuls (num, den), epilogue
            out = sig * num * reciprocal(den), with den evacuated from PSUM
            by the otherwise-idle ACT engine. The last i-chunk is split into
            two column halves so the final reciprocal overlaps its matmuls.
"""

import os
import sys

import ml_dtypes
import numpy as np

for _p in ("/opt/trn_rl_repo", "/root/.axon_site/_ro/trn_rl_repo"):
    if os.path.isdir(_p) and _p not in sys.path:
        sys.path.insert(0, _p)

import concourse.bass as bass
import concourse.tile as tile
from concourse import bacc, mybir
from concourse.bass_utils import run_bass_kernel_spmd

BS, N, D = 8, 2048, 512
P = 128
NCH = N // P  # 16 sequence chunks
KC = D // P  # 4 contraction chunks for projections
NB = 4  # xT column blocks (of 512) for startup pipelining
NWARM = 9
F32 = mybir.dt.float32
BF16 = mybir.dt.bfloat16
NP_BF16 = ml_dtypes.bfloat16

_NC_CACHE = {}


def _pbcast(ap_1xd, parts):
    """[1, D] dram AP -> partition-broadcast [parts, D] AP for DMA."""
    return bass.AP(
        tensor=ap_1xd.tensor, offset=ap_1xd.offset, ap=[[0, parts], ap_1xd.ap[1]]
    )


def build_nc():
    nc = bacc.Bacc("TRN2", target_bir_lowering=False, debug=False, num_devices=BS)

    xT = nc.dram_tensor("xT", [D, N], BF16, kind="ExternalInput").ap()
    wqkv = nc.dram_tensor("wqkv", [D, 3 * D], BF16, kind="ExternalInput").ap()
    bqf = nc.dram_tensor("bqf", [P, D], F32, kind="ExternalInput").ap()
    bvf = nc.dram_tensor("bvf", [P, D], F32, kind="ExternalInput").ap()
    ebt = nc.dram_tensor("ebt", [N, N], BF16, kind="ExternalInput").ap()
    out = nc.dram_tensor("out", [N, D], F32, kind="ExternalOutput").ap()

    # B^T viewed as [ji(=partition), jo, i]
    ebt_v = ebt.rearrange("(jo ji) i -> ji jo i", ji=P)

    with tile.TileContext(nc) as tc:
        with (
            tc.tile_pool(name="consts", bufs=1) as consts,
            tc.tile_pool(name="proj", bufs=1) as proj,
            tc.tile_pool(name="xpool", bufs=1) as xpool,
            tc.tile_pool(name="eqpool", bufs=1) as eqpool,
            tc.tile_pool(name="ebpool", bufs=2) as ebpool,
            tc.tile_pool(name="epi", bufs=2) as epi,
            tc.tile_pool(name="psum", bufs=2, space="PSUM") as psum,
        ):
            # ---- PE pre-warm: dependency-free matmuls on memset tiles raise
            # the HAM clock gate while the first input DMAs are in flight;
            # results are never consumed.
            warm_w = consts.tile([P, P], BF16, tag="warm_w")
            nc.gpsimd.memset(warm_w, 1.0)
            warm_r = consts.tile([P, D], BF16, tag="warm_r")
            nc.vector.memset(warm_r, 1.0)
            # two interleaved accumulation groups on alternating banks so
            # the warm matmuls stream back-to-back (same-bank back-to-back
            # matmuls serialize on the PSUM drain)
            warm_a = psum.tile([P, D], F32, tag="A", bufs=3)
            warm_b = psum.tile([P, D], F32, tag="B", bufs=3)
            half = NWARM // 2
            for w in range(half):
                nc.tensor.matmul(
                    warm_a, warm_w, warm_r,
                    start=(w == 0), stop=(w == half - 1),
                )
                nc.tensor.matmul(
                    warm_b, warm_w, warm_r,
                    start=(w == 0), stop=(w == half - 1),
                )

            # weights + input.T, batched into one DMA per projection /
            # per xT column block (few large issues instead of 30 small
            # ones: the Sync engine issues ~650ns per dma_start). Order
            # matches consumption: q weights + first xT block first.
            wqkv_v = wqkv.rearrange("(c p) n -> p c n", p=P)
            xT_v = xT.rearrange("(c p) n -> p c n", p=P)
            w_all = {}
            xt_b = {}

            def _dma_w(proj_i):
                w = proj.tile([P, KC, D], BF16, tag=f"w{proj_i}")
                nc.sync.dma_start(
                    w, wqkv_v[:, :, proj_i * D : (proj_i + 1) * D]
                )
                w_all[proj_i] = w

            def _dma_xt(b):
                x = proj.tile([P, KC, N // NB], BF16, tag=f"xt{b}")
                nc.sync.dma_start(
                    x, xT_v[:, :, b * (N // NB) : (b + 1) * (N // NB)]
                )
                xt_b[b] = x

            _dma_w(0)
            _dma_xt(0)
            # bias tiles, broadcast on the host so these are plain
            # contiguous reads; needed by the first q epilogue
            bq_bc = consts.tile([P, D], F32, tag="bq")
            nc.sync.dma_start(bq_bc, bqf)
            bv_bc = consts.tile([P, D], F32, tag="bv")
            nc.sync.dma_start(bv_bc, bvf)
            _dma_xt(1)
            _dma_xt(2)
            _dma_w(1)
            _dma_xt(3)
            _dma_w(2)
            w_t = {(pi, c): w_all[pi][:, c, :] for pi in range(3) for c in range(KC)}

            def lhs(n, c):
                b, r = divmod(n, NB)
                return xt_b[b][:, c, r * P : (r + 1) * P]

            # ---- phase 1a: q projection, sig = sigmoid(q+bq) ----
            sig_t = []
            for n in range(NCH):
                ps = psum.tile([P, D], F32, tag="A", bufs=3)
                for c in range(KC):
                    nc.tensor.matmul(
                        ps, lhs(n, c), w_t[0, c], start=(c == 0), stop=(c == KC - 1)
                    )
                qb = epi.tile([P, D], F32, tag="qb", bufs=3)
                nc.vector.tensor_add(qb, ps, bq_bc)
                sig = eqpool.tile([P, D], F32, tag=f"sig{n}")
                nc.scalar.activation(
                    sig, qb, mybir.ActivationFunctionType.Sigmoid
                )
                sig_t.append(sig)

            # ---- phase 1b: k, v projections; X = [ekv | ek] ----
            x_t = []
            for n in range(NCH):
                psk = psum.tile([P, D], F32, tag="A", bufs=3)
                psv = psum.tile([P, D], F32, tag="B", bufs=3)
                for c in range(KC):
                    nc.tensor.matmul(
                        psk, lhs(n, c), w_t[1, c], start=(c == 0), stop=(c == KC - 1)
                    )
                    nc.tensor.matmul(
                        psv, lhs(n, c), w_t[2, c], start=(c == 0), stop=(c == KC - 1)
                    )
                xt_tile = xpool.tile([P, 2 * D], BF16, tag=f"X{n}")
                nc.scalar.activation(
                    xt_tile[:, D : 2 * D], psk, mybir.ActivationFunctionType.Exp
                )
                vtmp = epi.tile([P, D], F32, tag="vtmp")
                nc.vector.tensor_add(vtmp, psv, bv_bc)
                nc.vector.tensor_mul(xt_tile[:, 0:D], vtmp, xt_tile[:, D : 2 * D])
                x_t.append(xt_tile)

            # ---- phase 2: per i-chunk big matmul + epilogue ----
            def epilogue(i, pd, pn, lo, hi):
                # out = sigmoid(q) * num / den
                w = hi - lo
                den = epi.tile([P, w], F32, tag="den")
                nc.scalar.copy(den, pd)
                rec = epi.tile([P, w], F32, tag="rec")
                nc.vector.reciprocal(rec, den)
                ob = epi.tile([P, w], F32, tag="ob")
                nc.vector.tensor_mul(ob, pn, rec)
                nc.vector.tensor_mul(ob, ob, sig_t[i][:, lo:hi])
                nc.sync.dma_start(out[i * P : (i + 1) * P, lo:hi], ob)

            GI = 8  # i-chunks per eb DMA group (2KB contiguous runs)
            eb_g = None
            for i in range(NCH):
                if i % GI == 0:
                    eb_g = ebpool.tile([P, NCH, GI * P], BF16, tag="eb")
                    nc.sync.dma_start(
                        eb_g, ebt_v[:, :, i * P : (i + GI) * P]
                    )
                eb = eb_g[:, :, (i % GI) * P : (i % GI + 1) * P]
                if i < NCH - 2:
                    pn = psum.tile([P, D], F32, tag="A", bufs=3)
                    pd = psum.tile([P, D], F32, tag="B", bufs=3)
                    for j in range(NCH):
                        nc.tensor.matmul(
                            pd, eb[:, j, :], x_t[j][:, D : 2 * D],
                            start=(j == 0), stop=(j == NCH - 1),
                        )
                        nc.tensor.matmul(
                            pn, eb[:, j, :], x_t[j][:, 0:D],
                            start=(j == 0), stop=(j == NCH - 1),
                        )
                    epilogue(i, pd, pn, 0, D)
                else:
                    # final two chunks: all den matmuls first (lo/hi halves
                    # interleaved across two banks), so both den epilogues
                    # (ACT copy + the slow reciprocals) fully overlap the
                    # num matmuls; only the final muls + DMA remain in the
                    # kernel tail
                    H = D // 2
                    pd_h = [
                        psum.tile([P, H], F32, tag="B", bufs=3, name="pd_lo"),
                        psum.tile([P, H], F32, tag="C", name="pd_hi"),
                    ]
                    for j in range(NCH):
                        for h in range(2):
                            nc.tensor.matmul(
                                pd_h[h], eb[:, j, :],
                                x_t[j][:, D + h * H : D + (h + 1) * H],
                                start=(j == 0), stop=(j == NCH - 1),
                            )
                    recs = []
                    for h in range(2):
                        den = epi.tile([P, H], F32, tag="den")
                        nc.scalar.copy(den, pd_h[h])
                        rec = epi.tile([P, H], F32, tag="rec")
                        nc.vector.reciprocal(rec, den)
                        recs.append(rec)
                    pn_h = [
                        psum.tile([P, H], F32, tag="A", bufs=3, name="pn_lo"),
                        psum.tile([P, H], F32, tag="A", bufs=3, name="pn_hi"),
                    ]
                    for j in range(NCH):
                        for h in range(2):
                            nc.tensor.matmul(
                                pn_h[h], eb[:, j, :],
                                x_t[j][:, h * H : (h + 1) * H],
                                start=(j == 0), stop=(j == NCH - 1),
                            )
                    for h in range(2):
                        ob = epi.tile([P, H], F32, tag="ob")
                        nc.vector.tensor_mul(ob, pn_h[h], recs[h])
                        nc.vector.tensor_mul(
                            ob, ob, sig_t[i][:, h * H : (h + 1) * H]
                        )
                        nc.sync.dma_start(
                            out[i * P : (i + 1) * P, h * H : (h + 1) * H], ob
                        )

    nc.compile()
    return nc


def get_nc():
    if "nc" not in _NC_CACHE:
        _NC_CACHE["nc"] = build_nc()
    return _NC_CACHE["nc"]


def prepare_in_maps(input, Wq, bq, Wk, bk, Wv, bv, pos_bias):
    input, Wq, bq, Wk, bk, Wv, bv, pos_bias = (
        np.asarray(a, dtype=np.float32)
        for a in (input, Wq, bq, Wk, bk, Wv, bv, pos_bias)
    )
    wqkv = np.concatenate([Wq, Wk, Wv], axis=1).astype(NP_BF16)
    bqf2 = np.ascontiguousarray(
        np.broadcast_to(bq.astype(np.float32), (P, D))
    )
    bvf2 = np.ascontiguousarray(
        np.broadcast_to(bv.astype(np.float32), (P, D))
    )
    ebt = np.exp(np.ascontiguousarray(pos_bias.T)).astype(NP_BF16)
    in_maps = []
    for b in range(BS):
        xT = np.ascontiguousarray(input[b].T).astype(NP_BF16)
        in_maps.append(
            {"xT": xT, "wqkv": wqkv, "bqf": bqf2, "bvf": bvf2, "ebt": ebt}
        )
    return in_maps


def kernel(input, Wq, bq, Wk, bk, Wv, bv, pos_bias, _run_kwargs=None):
    nc = get_nc()
    in_maps = prepare_in_maps(input, Wq, bq, Wk, bk, Wv, bv, pos_bias)
    res = run_bass_kernel_spmd(
        nc, in_maps, core_ids=list(range(BS)), **(_run_kwargs or {})
    )
    out = np.stack([res.results[b]["out"] for b in range(BS)], axis=0)
    if _run_kwargs:
        kernel.last_results = res
    return out

